# revision 40
# baseline (speedup 1.0000x reference)
"""Masked causal self-attention (single head) on 8 Trainium2 NeuronCores.

Problem: x[4,4096,1024], mask[4,4096] (key padding), Wq/Wk/Wv[128,1024],
bq/bk/bv[128] -> out[4,4096,128]:
    q = x@Wq.T+bq; k = x@Wk.T+bk; v = x@Wv.T+bv
    out = softmax(causal_mask(q@k.T/sqrt(128)) + key_padding) @ v

Sharding (SPMD, one program on 8 cores): core c = (batch b=c//2, parity
p=c%2). Each core computes K/V for its full batch (replicated within the
pair) and handles the interleaved query 128-row tiles {2*t+p : t in 0..15}
- interleaving balances the causal (triangular) work between the pair.

All PE operands are bf16 (the PE computes at FP22 for both fp32r and bf16 so
this costs only storage precision ~0.4%; bf16 moving operands stream at
2 elem/cycle and Fast-Weight-Load halves the LDWEIGHTS cost).

Host-side packing (per batch, per parity):
  - xp[128, sc(8)*dc(8)*pos(4)*128] bf16: x^T tiled so one dma_start
    delivers [128, 4dc, 512] with d-chunk on partitions and 4KB/partition
    contiguous bursts. Within each 512 s-chunk the four 128-subtiles are
    permuted by sigma_p ([0,1,2,3] for parity 0, [1,0,3,2] for parity 1) so
    the core's OWNED q-subtiles sit at fixed positions 0 and 2 - this makes
    the single SPMD program parity-independent (Q is projected from the same
    x tiles with a strided moving AP; no separate xq input). K/V/score
    k-tile order is this permuted order; the key-padding mask and the causal
    0/1 patterns (patt) are host-built in the same order.
  - 1/sqrt(128) folded into Wq/bq on the host.

Key padding is NOT applied to the scores: masked keys are instead excluded
from both the numerator (v_t rows scaled by the 0/1 key mask during PSUM
eviction - same DVE cost as the plain copy) and the denominator (the den
matmul's stationary is a host-built mask-replicated tile instead of ones).
This frees the exp of any per-k-tile bias, so one scalar-engine activation
handles TWO k-tiles [128, 2*512] at once, amortizing the ~352-cycle
activation overhead (the scalar engine at 1.2 GHz is the co-bottleneck).

Device schedule per core: projections run two 512-query blocks ahead of
attention so the PE never stalls on the Q eviction at a block boundary, and
the attention blocks run in order 1,2,3,0 so the smallest block (8 k-tiles)
drains the pipeline at the end:
  proj0 proj1 q0 proj2 proj3 q1 proj4 proj5 q2 attn1 proj6 proj7 q3
  attn2 attn3 attn0
Within attention: scores^T[k,q] per pair of k-tiles -> one exp -> causal
0/1-pattern bf16 multiply (diag pairs only) -> outT[h,q] += v_t[kt].T @ PT
and den += mrep[kt].T @ PT accumulated in PSUM; epilogue reciprocal +
multiply; output [H, NQ] f32 DMA'd on the GpSimd queue, host transposes.

Startup: 24 dummy ident@ones matmuls warm the PE HAM clock-gate (cold PE
runs at 1.2 GHz for ~3.4us) and a dummy exp preloads the scalar engine's
activation table (~2.7us one-time) while the first DMAs land.
"""

import sys

sys.path.insert(0, "/opt/trn_rl_repo")

import numpy as np
import ml_dtypes

import concourse.bass as bass
import concourse.bacc as bacc
import concourse.tile as tile
from concourse import mybir
from concourse.masks import make_identity
from concourse import bass_utils

F32 = mybir.dt.float32
BF16 = mybir.dt.bfloat16
FP8 = mybir.dt.float8e4
FP8E3 = mybir.dt.float8e3
BF16_NP = ml_dtypes.bfloat16
FP8_NP = ml_dtypes.float8_e4m3fn
E3M4_NP = ml_dtypes.float8_e3m4
WSCALE = 32.0
QSC = 1.0 / (WSCALE * float(np.sqrt(128.0)))
B, S, D, H = 4, 4096, 1024, 128
NQ = S // 2          # queries owned per core (2048)
DC = D // 128        # 8 d-chunks
SCH = S // 512       # 8 seq chunks of 512
NKT = S // 128       # 32 key tiles
ATTN_ORDER = (1, 2, 3, 0)


def _build_program():
    nc = bacc.Bacc("TRN2", target_bir_lowering=False)

    xp_d = nc.dram_tensor("xp", [128, SCH * DC * 512], BF16, kind="ExternalInput")
    xp8_d = nc.dram_tensor("xp8", [128, SCH * DC * 512], FP8, kind="ExternalInput")
    wv8_d = nc.dram_tensor("wv8", [128, DC * H], FP8, kind="ExternalInput")
    wq16_d = nc.dram_tensor("wq16", [128, DC * H], BF16, kind="ExternalInput")
    wk16_d = nc.dram_tensor("wk16", [128, DC * H], BF16, kind="ExternalInput")
    wv_d = nc.dram_tensor("wv", [128, DC * H], BF16, kind="ExternalInput")
    bq_d = nc.dram_tensor("bq", [H, 1], F32, kind="ExternalInput")
    bk_d = nc.dram_tensor("bk", [H, 1], F32, kind="ExternalInput")
    bv_d = nc.dram_tensor("bv", [H, 1], F32, kind="ExternalInput")
    vm_d = nc.dram_tensor("vmask", [128, NKT], F32, kind="ExternalInput")
    mr_d = nc.dram_tensor("mrep", [128, NKT * H], FP8, kind="ExternalInput")
    pt_d = nc.dram_tensor("patt", [128, 8 * 512], BF16, kind="ExternalInput")
    o_d = nc.dram_tensor("o", [H, NQ], F32, kind="ExternalOutput")

    with tile.TileContext(nc) as tc:
        with (
            tc.tile_pool(name="consts", bufs=1) as consts,
            tc.tile_pool(name="big", bufs=1) as big,
            tc.tile_pool(name="vtiles", bufs=NKT) as vtiles,
            tc.tile_pool(name="ptp", bufs=6) as ptp,
            tc.tile_pool(name="pt2p", bufs=4) as pt2p,
        ):
            # ---- engine-generated constants (no DMA dependence) ----
            ident = consts.tile([128, 128], BF16)
            make_identity(nc, ident)
            ones = consts.tile([128, 128], BF16, tag="ones")
            nc.vector.memset(ones, 1.0)
            neg1 = consts.tile([128, 1], F32, tag="neg1")
            nc.vector.memset(neg1, -1.0)
            act_warm = consts.tile([128, 1], F32, tag="act_warm")
            nc.scalar.activation(
                act_warm, ident[:, 0:1], mybir.ActivationFunctionType.Exp,
                bias=neg1)

            # ---- input DMAs: weights/consts on the gpsimd queue ----
            w_sb = {}
            for name, dram in (("k16", wk16_d), ("v", wv_d), ("q16", wq16_d)):
                t = consts.tile([128, DC, H], BF16, tag=f"w_{name}", name="wt")
                nc.gpsimd.dma_start(
                    out=t, in_=dram[:, :].rearrange("p (c h) -> p c h", c=DC))
                w_sb[name] = t
            wv8 = consts.tile([128, 4, 2, H], FP8, tag="w_v8")
            nc.gpsimd.dma_start(
                out=wv8, in_=wv8_d[:, :].rearrange(
                    "p (c o h) -> p c o h", c=4, o=2))
            w_sb["v8"] = wv8
            b_sb = {}
            for name, dram in (("k", bk_d), ("v", bv_d), ("q", bq_d)):
                t = consts.tile([H, 1], F32, tag=f"b_{name}")
                nc.gpsimd.dma_start(out=t, in_=dram[:, :])
                b_sb[name] = t
            vm = consts.tile([128, NKT], F32, tag="vm")
            nc.gpsimd.dma_start(out=vm, in_=vm_d[:, :])
            mrep = consts.tile([128, NKT, H], FP8, tag="mrep")
            patt = consts.tile([128, 8, 512], BF16, tag="patt")

            def load_small_consts():
                nc.gpsimd.dma_start(
                    out=mrep, in_=mr_d[:, :].rearrange("p (k h) -> p k h", k=NKT))
                nc.gpsimd.dma_start(
                    out=pt_d_sb, in_=pt_d[:, :].rearrange("p (r s) -> p r s", r=8))
            pt_d_sb = patt

            KT = big.tile([128, S], BF16, tag="KT")     # K^T [h, k]
            QT = big.tile([128, NQ], BF16, tag="QT")    # Q^T [h, q]

            # ---- PE/HAM warmup: back-to-back dummy matmuls ----
            with tc.tile_pool(name="warm", bufs=2, space="PSUM") as warm:
                for _ in range(14):
                    wp = warm.tile([128, 128], F32)
                    nc.tensor.matmul(wp, ident, ones, start=True, stop=True)

            # ---- interleaved projections + attention ----
            with (
                tc.tile_pool(name="vt_sb", bufs=1) as vt_sb_pool,
                tc.tile_pool(name="xp", bufs=16) as xpp,
                tc.tile_pool(name="x8", bufs=8) as x8p,
                tc.tile_pool(name="pp", bufs=2, space="PSUM") as pp,
                tc.tile_pool(name="sp", bufs=2, space="PSUM") as sp,
                tc.tile_pool(name="op", bufs=1, space="PSUM") as op,
                tc.tile_pool(name="dp", bufs=1, space="PSUM") as dp,
                tc.tile_pool(name="osb", bufs=2) as osb,
                tc.tile_pool(name="rp", bufs=2) as rp,
            ):
                VT = vt_sb_pool.tile([128, S], BF16, tag="VT")
                v_t = [None] * NKT
                kv_ps = {}
                q_ps = {}
                x_sb = {}   # (sc, half) -> SBUF tile [128, 4, 512] bf16
                x8_sb = {}  # sc -> SBUF tile [128, 4, 2, 512] e3m4

                def load_x(sc, dh):
                    xt = xpp.tile([128, 4, 512], BF16, tag="xt")
                    lo = sc * (DC * 512) + dh * (4 * 512)
                    nc.sync.dma_start(
                        out=xt,
                        in_=xp_d[:, lo:lo + 4 * 512].rearrange(
                            "p (c s) -> p c s", c=4))
                    x_sb[(sc, dh)] = xt

                def load_x8(sc):
                    xt = x8p.tile([128, 4, 2, 512], FP8, tag="xt8")
                    lo = sc * (DC * 512)
                    nc.sync.dma_start(
                        out=xt,
                        in_=xp8_d[:, lo:lo + DC * 512].rearrange(
                            "p (c o s) -> p c o s", c=4, o=2))
                    x8_sb[sc] = xt

                def k_project(kpsum, sc, hi_prec):
                    for dh in range(2):
                        xt = x_sb[(sc, dh)]
                        for dcl in range(4):
                            dc = dh * 4 + dcl
                            nc.tensor.matmul(
                                kpsum, w_sb["k16"][:, dc, :],
                                xt[:, dcl, :],
                                start=(dc == 0), stop=(dc == DC - 1))

                def k_evict(kpsum, sc, hi_prec):
                    nc.vector.tensor_scalar_add(
                        KT[:, sc * 512:(sc + 1) * 512], kpsum, b_sb["k"])

                def v_project(vpsum, sc):
                    # scores are fp8-sensitive but V is not: value errors
                    # enter the output linearly and average over the softmax,
                    # except for the short-context rows which only read
                    # v-tiles 0-7 -> s-chunks 0,1 project in bf16, the rest
                    # via fp8 DoubleRow
                    if sc < 2:
                        for dh in range(2):
                            xt = x_sb[(sc, dh)]
                            for dcl in range(4):
                                dc = dh * 4 + dcl
                                nc.tensor.matmul(
                                    vpsum, w_sb["v"][:, dc, :], xt[:, dcl, :],
                                    start=(dc == 0), stop=(dc == DC - 1))
                    else:
                        x8t = x8_sb[sc]
                        for dcp in range(4):
                            nc.tensor.matmul(
                                vpsum, w_sb["v8"][:, dcp, :, :],
                                x8t[:, dcp, :, :],
                                start=(dcp == 0), stop=(dcp == 3),
                                perf_mode=mybir.MatmulPerfMode.DoubleRow)

                def v_evict(vpsum, sc):
                    if sc < 2:
                        nc.vector.tensor_scalar_add(
                            VT[:, sc * 512:(sc + 1) * 512], vpsum, b_sb["v"])
                    else:
                        nc.vector.tensor_scalar(
                            VT[:, sc * 512:(sc + 1) * 512], vpsum,
                            1.0 / WSCALE, b_sb["v"],
                            mybir.AluOpType.mult, mybir.AluOpType.add)

                def project_sc(sc):
                    hi = sc < 2
                    kpsum = pp.tile([128, 512], F32, tag="pj")
                    vpsum = pp.tile([128, 512], F32, tag="pj")
                    k_project(kpsum, sc, hi)
                    v_project(vpsum, sc)
                    k_evict(kpsum, sc, hi)
                    v_evict(vpsum, sc)
                    for kt in range(4 * sc, 4 * sc + 4):
                        tpsum = pp.tile([128, 512], F32, tag="pj")
                        tview = tpsum.bitcast(BF16)[:, 0:128]
                        nc.tensor.transpose(
                            tview, VT[:, kt * 128:(kt + 1) * 128], ident)
                        vt = vtiles.tile([128, H], BF16, tag="v_t")
                        # fold the 0/1 key-padding mask into V^T rows
                        nc.vector.tensor_scalar_mul(vt, tview, vm[:, kt:kt + 1])
                        v_t[kt] = vt

                def owned_cols16(xt, dcl):
                    # bf16 moving AP over owned subtiles (positions 0, 2)
                    row = xt[:, dcl, :]
                    return bass.AP(
                        tensor=row.tensor, offset=row.offset,
                        ap=[list(row.ap[0]), [256, 2], [1, 128]])

                def owned_cols8(x8t, dcp):
                    # moving AP over the core's two owned 128-subtiles
                    # (positions 0 and 2) of both chunks of the dc-pair:
                    # [128, 2(dc), 2(block), 128]
                    row = x8t[:, dcp, :, :]
                    return bass.AP(
                        tensor=row.tensor, offset=row.offset,
                        ap=[list(row.ap[0]), [512, 2], [256, 2], [1, 128]])

                def q_project(qpsum, j):
                    for e in range(2):
                        for dh in range(2):
                            xt = x_sb[(2 * j + e, dh)]
                            for dcl in range(4):
                                dc = dh * 4 + dcl
                                nc.tensor.matmul(
                                    qpsum[:, e * 256:(e + 1) * 256],
                                    w_sb["q16"][:, dc, :],
                                    owned_cols16(xt, dcl),
                                    start=(dc == 0), stop=(dc == DC - 1))

                def q_evict(j):
                    nc.vector.tensor_scalar_add(
                        QT[:, j * 512:(j + 1) * 512], q_ps[j], b_sb["q"])

                def project_q(j):
                    qpsum = pp.tile([128, 512], F32, tag="pj")
                    q_ps[j] = qpsum
                    q_project(qpsum, j)
                    q_evict(j)

                pending = []

                def drain(n):
                    for _ in range(min(n, len(pending))):
                        pending.pop(0)()

                def attention(j, rate=2):
                    n_kt = 8 * j + 8
                    n_pr = n_kt // 2
                    outp = op.tile([128, 512], F32)
                    denp = dp.tile([128, 512], F32)
                    pts = [None] * n_pr

                    def acol(pr):
                        # causal dead-prefix: diagonal pair w of the chunk
                        # has its first w*128 query columns fully masked
                        # (valid for both parities)
                        return 128 * max(0, pr - 4 * j)

                    def score_exp(pr):
                        a = acol(pr)
                        spair = sp.tile([128, 2, 512], F32)
                        for o in range(2):
                            kt = 2 * pr + o
                            nc.tensor.matmul(
                                spair[:, o, a:512],
                                KT[:, kt * 128:(kt + 1) * 128],
                                QT[:, j * 512 + a:(j + 1) * 512],
                                start=True, stop=True)
                        pt = ptp.tile([128, 2, 512], BF16, tag="pt")
                        # bias -1 keeps exp within fp8e4 range (max score
                        # ~5.5 sigma); it scales num and den equally so the
                        # softmax ratio is unchanged
                        nc.scalar.activation(
                            pt[:, :, a:512], spair[:, :, a:512],
                            mybir.ActivationFunctionType.Exp, bias=neg1)
                        if 2 * pr >= 8 * j:
                            r = 2 * pr - 8 * j
                            pt2 = pt2p.tile([128, 2, 512], BF16, tag="pt2")
                            nc.vector.tensor_mul(
                                pt2[:, :, a:512], pt[:, :, a:512],
                                patt[:, r:r + 2, a:512])
                            pts[pr] = pt2
                        else:
                            pts[pr] = pt

                    def pv(pr):
                        a = acol(pr)
                        for o in range(2):
                            kt = 2 * pr + o
                            nc.tensor.matmul(
                                outp[:, a:512], v_t[kt], pts[pr][:, o, a:512],
                                start=(kt == 0), stop=(kt == n_kt - 1))
                        for o in range(2):
                            kt = 2 * pr + o
                            nc.tensor.matmul(
                                denp[:, a:512], mrep[:, kt, :],
                                pts[pr][:, o, a:512],
                                start=(kt == 0), stop=(kt == n_kt - 1))

                    # software-pipelined: PE does scores(pr+1) while the
                    # scalar engine exps scores(pr); PV lags one step.
                    # projection work for later blocks is drained into the
                    # exp-latency bubbles between steps.
                    score_exp(0)
                    for pr in range(1, n_pr):
                        score_exp(pr)
                        drain(rate)
                        pv(pr - 1)
                        drain(rate)
                    pv(n_pr - 1)

                    r_t = rp.tile([128, 512], F32, tag="r")
                    nc.vector.reciprocal(r_t, denp)
                    o_sb = osb.tile([128, 512], F32, tag="o")
                    nc.vector.tensor_mul(o_sb, outp, r_t)
                    nc.sync.dma_start(
                        out=o_d[:, j * 512:(j + 1) * 512], in_=o_sb)

                def queue_proj(sc):
                    def k_mms(sc=sc):
                        k_project(kv_ps[sc][0], sc, sc < 2)

                    def v_mms(sc=sc):
                        v_project(kv_ps[sc][1], sc)

                    def alloc(sc=sc):
                        kv_ps[sc] = (
                            pp.tile([128, 512], F32, tag="pj", name="kpsum"),
                            pp.tile([128, 512], F32, tag="pj", name="vpsum"))
                    pending.append(alloc)
                    pending.append(k_mms)
                    pending.append(v_mms)

                    def evict(sc=sc):
                        kpsum, vpsum = kv_ps[sc]
                        k_evict(kpsum, sc, sc < 2)
                        v_evict(vpsum, sc)
                    pending.append(evict)

                    def transp(kt):
                        tpsum = pp.tile([128, 512], F32, tag="pj")
                        tview = tpsum.bitcast(BF16)[:, 0:128]
                        nc.tensor.transpose(
                            tview, VT[:, kt * 128:(kt + 1) * 128], ident)
                        vt = vtiles.tile([128, H], BF16, tag="v_t")
                        nc.vector.tensor_scalar_mul(vt, tview, vm[:, kt:kt + 1])
                        v_t[kt] = vt
                    for kt in range(4 * sc, 4 * sc + 4):
                        pending.append(lambda kt=kt: transp(kt))

                def queue_q(j):
                    def alloc(j=j):
                        q_ps[j] = pp.tile(
                            [128, 512], F32, tag="pj", name="qpsum")

                    def q_mms(j=j):
                        q_project(q_ps[j], j)

                    def evict(j=j):
                        q_evict(j)
                    pending.append(alloc)
                    pending.append(q_mms)
                    pending.append(evict)

                # prefetch the whole x stream (the DMA queue runs in order)
                for sc in range(2):
                    load_x(sc, 0)
                    load_x(sc, 1)
                for sc in (2, 3):
                    load_x8(sc)
                    load_x(sc, 0)
                    load_x(sc, 1)
                load_small_consts()
                for sc in range(4, SCH):
                    load_x8(sc)
                    load_x(sc, 0)
                    load_x(sc, 1)
                project_sc(0)
                project_q(0)
                project_sc(1)
                # Q projections only need x tiles, so they go FIRST in each
                # drain batch: the QT eviction clears the DVE queue early and
                # the next attention block starts without stalling.
                queue_q(1)
                for sc in (2, 3):
                    queue_proj(sc)
                attention(0)
                drain(len(pending))
                queue_q(2)
                for sc in (4, 5):
                    queue_proj(sc)
                attention(1)
                drain(len(pending))
                queue_q(3)
                for sc in (6, 7):
                    queue_proj(sc)
                attention(2)
                drain(len(pending))
                attention(3)
    nc.compile()
    return nc


_NC_CACHE = {}


def _get_program():
    if "nc" not in _NC_CACHE:
        _NC_CACHE["nc"] = _build_program()
    return _NC_CACHE["nc"]


def _sigma(p):
    # within-chunk subtile permutation: owned subtiles at positions 0, 2
    return [0, 1, 2, 3] if p == 0 else [1, 0, 3, 2]


def _make_in_maps(x, mask, Wq, bq, Wk, bk, Wv, bv):
    x = np.asarray(x, np.float32)
    mask = np.asarray(mask)
    scale = 1.0 / np.sqrt(np.float32(H))

    def pack_w(w):
        # [H,D] -> w.T [D,H] -> partition-major [128, DC*H]
        wT = np.asarray(w, np.float32).T.reshape(DC, 128, H)
        return np.ascontiguousarray(
            wT.transpose(1, 0, 2).reshape(128, DC * H).astype(BF16_NP))

    def pack_w8(w):
        # [H,D] -> WSCALE*w.T in e3m4, laid out [128, dcp(4), o(2), H]
        wT = (np.asarray(w, np.float32).T * WSCALE).clip(-200.0, 200.0)
        wT = wT.reshape(4, 2, 128, H)           # [dcp, o, dp, h]
        return np.ascontiguousarray(
            wT.transpose(2, 0, 1, 3).reshape(128, DC * H).astype(FP8_NP))

    wv8 = pack_w8(Wv)
    wq16 = pack_w(np.asarray(Wq, np.float32) * scale)
    wk16 = pack_w(Wk)
    wv = pack_w(Wv)
    bq_c = (np.asarray(bq, np.float32) * scale).reshape(H, 1).copy()
    bk_c = np.asarray(bk, np.float32).reshape(H, 1).copy()
    bv_c = np.asarray(bv, np.float32).reshape(H, 1).copy()

    # per (batch, parity) packed x^T: [dp, sc, dc, pos, sf]
    xp_cache = {}

    def pack_x(b, p):
        if (b, p) not in xp_cache:
            xb = x[b].astype(BF16_NP)               # [s, d]
            xr = xb.reshape(SCH, 4, 128, DC, 128)   # [sc, t, sf, dc, dp]
            xr = xr[:, _sigma(p), :, :, :]          # [sc, pos, sf, dc, dp]
            xp_cache[(b, p)] = np.ascontiguousarray(
                xr.transpose(4, 0, 3, 1, 2).reshape(128, SCH * DC * 512))
        return xp_cache[(b, p)]

    xp8_cache = {}

    def pack_x8(b, p):
        # [dp, sc, dcp, o, pos, sf] in e3m4 for the DoubleRow projections
        if (b, p) not in xp8_cache:
            xb = x[b].clip(-200.0, 200.0).astype(FP8_NP)
            xr = xb.reshape(SCH, 4, 128, 4, 2, 128)  # [sc,t,sf,dcp,o,dp]
            xr = xr[:, _sigma(p), :, :, :, :]
            xp8_cache[(b, p)] = np.ascontiguousarray(
                xr.transpose(5, 0, 3, 4, 1, 2).reshape(128, SCH * DC * 512))
        return xp8_cache[(b, p)]

    patt_cache = {}

    def pack_patt(p):
        # patt[kp, r, i*128+qf]: causal 0/1 for diagonal k-tile offset r
        if p not in patt_cache:
            sig = _sigma(p)
            kp = np.arange(128)[:, None, None, None]
            r = np.arange(8)[None, :, None, None]
            i = np.arange(4)[None, None, :, None]
            qf = np.arange(128)[None, None, None, :]
            koff = 4 * (r // 4) + np.array(sig)[r % 4]
            qoff = 2 * i + p
            m = (qoff > koff) | ((qoff == koff) & (qf >= kp))
            patt_cache[p] = np.ascontiguousarray(
                m.astype(BF16_NP).reshape(128, 8 * 512))
        return patt_cache[p]

    in_maps = []
    for c in range(8):
        b, p = c // 2, c % 2
        sig = np.array(_sigma(p))
        kt = np.arange(NKT)
        g_kt = 4 * (kt // 4) + sig[kt % 4]          # global tile of k-tile kt
        key_idx = g_kt[None, :] * 128 + np.arange(128)[:, None]
        kmask = (np.asarray(mask[b])[key_idx] != 0)         # [128, NKT]
        vmask = kmask.astype(np.float32)
        mrep = np.broadcast_to(
            kmask.astype(FP8_NP)[:, :, None], (128, NKT, H))
        in_maps.append({
            "xp": pack_x(b, p), "xp8": pack_x8(b, p),
            "wv8": wv8, "wv": wv, "wq16": wq16, "wk16": wk16,
            "bq": bq_c, "bk": bk_c, "bv": bv_c,
            "vmask": np.ascontiguousarray(vmask),
            "mrep": np.ascontiguousarray(mrep.reshape(128, NKT * H)),
            "patt": pack_patt(p),
        })
    return in_maps


def _install_ntff_hook():
    # the trimmed antenv package lacks axon_hooks; recreate it and wire the
    # ctypes NTFF profiling hook from trn_agent_boot so trace=True works
    import types
    if "antenv.axon_hooks" in sys.modules:
        return
    import antenv
    mod = types.ModuleType("antenv.axon_hooks")
    _hook = [None]
    mod.set_axon_ntff_profile_hook = lambda h: _hook.__setitem__(0, h)
    mod.get_axon_ntff_profile_hook = lambda: _hook[0]
    sys.modules["antenv.axon_hooks"] = mod
    antenv.axon_hooks = mod
    from trn_agent_boot.trn_boot import _ntff_profile_via_ctypes
    mod.set_axon_ntff_profile_hook(
        _ntff_profile_via_ctypes("/opt/axon/libaxon_pjrt.so"))


def run(inputs, trace=False, tmpdir=None):
    if trace:
        try:
            _install_ntff_hook()
        except Exception as e:
            print("ntff hook install failed:", e)
    nc = _get_program()
    in_maps = _make_in_maps(**inputs)
    res = bass_utils.run_bass_kernel_spmd(
        nc, in_maps, core_ids=list(range(8)), trace=trace, tmpdir=tmpdir)
    out = np.empty((B, S, H), np.float32)
    for c in range(8):
        b, p = c // 2, c % 2
        o = res.results[c]["o"]                                # [H, NQ]
        for lt in range(16):
            g = 2 * lt + p
            out[b, g * 128:(g + 1) * 128, :] = o[:, lt * 128:(lt + 1) * 128].T
    return out, res


def kernel(**inputs) -> np.ndarray:
    out, _ = run(inputs, trace=False)
    return out


# revision 42
# speedup vs baseline: 1.1596x; 1.1596x over previous
"""Masked causal self-attention (single head) on 8 Trainium2 NeuronCores.

Problem: x[4,4096,1024], mask[4,4096] (key padding), Wq/Wk/Wv[128,1024],
bq/bk/bv[128] -> out[4,4096,128]:
    q = x@Wq.T+bq; k = x@Wk.T+bk; v = x@Wv.T+bv
    out = softmax(causal_mask(q@k.T/sqrt(128)) + key_padding) @ v

Sharding (SPMD, one program on 8 cores): core c = (batch b=c//2, parity
p=c%2). Each core computes K/V for its full batch (replicated within the
pair) and handles the interleaved query 128-row tiles {2*t+p : t in 0..15}
- interleaving balances the causal (triangular) work between the pair.

All PE operands are bf16 or fp8 (the PE computes at FP22 for fp32r/bf16
alike, so bf16 costs only storage precision ~0.4%, enables Fast-Weight-Load,
and halves DMA bytes; the LDWEIGHTS then hides completely under the previous
matmul's streaming, leaving the PE at its 1 elem/cycle streaming floor).

Host-side packing (per batch, per parity):
  - xp[128, sc(8)*dc(8)*pos(4)*128] bf16: x^T tiled so one dma_start
    delivers [128, 4dc, 512] with d-chunk on partitions and 4KB/partition
    contiguous bursts. Within each 512 s-chunk the four 128-subtiles are
    permuted by sigma_p ([0,1,2,3] for parity 0, [1,0,3,2] for parity 1) so
    the core's OWNED q-subtiles sit at fixed positions 0 and 2 - this makes
    the single SPMD program parity-independent (Q is projected from the same
    x tiles with a strided moving AP; no separate xq input). K/V/score
    k-tile order is this permuted order; the key-padding mask and the causal
    0/1 patterns (patt) are host-built in the same order.
  - 1/sqrt(128) folded into Wq/bq on the host.

Key padding is NOT applied to the scores: masked keys are instead excluded
from both the numerator (v_t rows scaled by the 0/1 key mask during PSUM
eviction - same DVE cost as the plain copy) and the denominator (the den
matmul's stationary is a host-built mask-replicated fp8 tile instead of
ones). This frees the exp of any per-k-tile bias, so one scalar-engine
activation handles TWO k-tiles [128, 2, 512] at once, amortizing the
~352-cycle activation overhead (the 1.2 GHz scalar engine co-limits the
attention phases).

fp8 (e4m3) is used where precision analysis allows: the V projection for
s-chunks 2..7 runs as fp8 DoubleRow matmuls over d-chunk pairs (2 MACs/
cycle) - V errors enter the output linearly and average across >=1024-key
softmaxes, while short-context rows only read v-tiles 0-7 which stay bf16.
Scores (Q@K) must stay bf16: fp8 score noise is amplified through exp and
measured 3-6e-2 relative error. fp8 weights are scaled x32 on the host
(avoids e4m3 subnormals) and the PSUM eviction folds the 1/32 back in.
Engine-written fp8 (exp output or DVE copies) produces NaN on this stack,
so only DMA-written fp8 operands are used.

Causal dead-prefix trim: within attention block j, diagonal k-tile pair w
has its first w*128 query columns fully masked for both parities, so the
score/exp/mask/PV/den work is sliced to columns [w*128:512] (~20% of the
attention work removed).

Schedule (PE queue is FIFO, so emission order is execution order): x/x8
tiles are prefetched in need-order on the sync HWDGE ring while consts ride
the gpsimd ring; 14 dummy ident@ones matmuls warm the PE HAM clock-gate
(cold PE runs at 1.2 GHz) and a dummy exp preloads the scalar activation
table, both while the first DMAs land. Projections for later blocks are
queued as closures and drained two-at-a-time into the exp-latency bubbles
between attention steps:
  proj0 q0 proj1 | attn0{q1 proj2 proj3} attn1{q2 proj4 proj5}
  attn2{q3 proj6 proj7} attn3
(Q closures first in each batch so the QT eviction clears the DVE queue
before the next attention head needs it.)
Within attention: scores^T[k,q] per k-tile pair -> one exp -> causal
0/1-pattern multiply on DVE (diag pairs only) -> outT[h,q] += v_t[kt].T @ PT
and den += mrep[kt].T @ PT accumulated in PSUM; epilogue reciprocal +
multiply; output [H, NQ] f32, host transposes.
"""

import sys

sys.path.insert(0, "/opt/trn_rl_repo")

import numpy as np
import ml_dtypes

import concourse.bass as bass
import concourse.bacc as bacc
import concourse.tile as tile
from concourse import mybir
from concourse.masks import make_identity
from concourse import bass_utils

F32 = mybir.dt.float32
BF16 = mybir.dt.bfloat16
FP8 = mybir.dt.float8e4
BF16_NP = ml_dtypes.bfloat16
FP8_NP = ml_dtypes.float8_e4m3fn
WSCALE = 32.0
B, S, D, H = 4, 4096, 1024, 128
NQ = S // 2          # queries owned per core (2048)
DC = D // 128        # 8 d-chunks
SCH = S // 512       # 8 seq chunks of 512
NKT = S // 128       # 32 key tiles


def _build_program():
    nc = bacc.Bacc("TRN2", target_bir_lowering=False)

    xp_d = nc.dram_tensor("xp", [128, SCH * DC * 512], BF16, kind="ExternalInput")
    xp8_d = nc.dram_tensor("xp8", [128, SCH * DC * 512], FP8, kind="ExternalInput")
    wv8_d = nc.dram_tensor("wv8", [128, DC * H], FP8, kind="ExternalInput")
    wq16_d = nc.dram_tensor("wq16", [128, DC * H], BF16, kind="ExternalInput")
    wk16_d = nc.dram_tensor("wk16", [128, DC * H], BF16, kind="ExternalInput")
    wv_d = nc.dram_tensor("wv", [128, DC * H], BF16, kind="ExternalInput")
    bq_d = nc.dram_tensor("bq", [H, 1], F32, kind="ExternalInput")
    bk_d = nc.dram_tensor("bk", [H, 1], F32, kind="ExternalInput")
    bv_d = nc.dram_tensor("bv", [H, 1], F32, kind="ExternalInput")
    vm_d = nc.dram_tensor("vmask", [128, NKT], F32, kind="ExternalInput")
    mr_d = nc.dram_tensor("mrep", [128, NKT * H], FP8, kind="ExternalInput")
    pt_d = nc.dram_tensor("patt", [128, 8 * 512], BF16, kind="ExternalInput")
    o_d = nc.dram_tensor("o", [H, NQ], F32, kind="ExternalOutput")

    with tile.TileContext(nc) as tc:
        with (
            tc.tile_pool(name="consts", bufs=1) as consts,
            tc.tile_pool(name="big", bufs=1) as big,
            tc.tile_pool(name="vtiles", bufs=NKT) as vtiles,
            tc.tile_pool(name="ptp", bufs=6) as ptp,
            tc.tile_pool(name="pt2p", bufs=4) as pt2p,
        ):
            # ---- engine-generated constants (no DMA dependence) ----
            ident = consts.tile([128, 128], BF16)
            make_identity(nc, ident)
            ones = consts.tile([128, 128], BF16, tag="ones")
            nc.vector.memset(ones, 1.0)
            neg1 = consts.tile([128, 1], F32, tag="neg1")
            nc.vector.memset(neg1, -1.0)
            act_warm = consts.tile([128, 1], F32, tag="act_warm")
            nc.scalar.activation(
                act_warm, ident[:, 0:1], mybir.ActivationFunctionType.Exp,
                bias=neg1)

            # ---- input DMAs: weights/consts on the gpsimd queue ----
            w_sb = {}
            for name, dram in (("k16", wk16_d), ("v", wv_d), ("q16", wq16_d)):
                t = consts.tile([128, DC, H], BF16, tag=f"w_{name}", name="wt")
                nc.gpsimd.dma_start(
                    out=t, in_=dram[:, :].rearrange("p (c h) -> p c h", c=DC))
                w_sb[name] = t
            wv8 = consts.tile([128, 4, 2, H], FP8, tag="w_v8")
            nc.gpsimd.dma_start(
                out=wv8, in_=wv8_d[:, :].rearrange(
                    "p (c o h) -> p c o h", c=4, o=2))
            w_sb["v8"] = wv8
            b_sb = {}
            for name, dram in (("k", bk_d), ("v", bv_d), ("q", bq_d)):
                t = consts.tile([H, 1], F32, tag=f"b_{name}")
                nc.gpsimd.dma_start(out=t, in_=dram[:, :])
                b_sb[name] = t
            vm = consts.tile([128, NKT], F32, tag="vm")
            nc.gpsimd.dma_start(out=vm, in_=vm_d[:, :])
            mrep = consts.tile([128, NKT, H], FP8, tag="mrep")
            patt = consts.tile([128, 8, 512], BF16, tag="patt")

            def load_small_consts():
                nc.gpsimd.dma_start(
                    out=mrep, in_=mr_d[:, :].rearrange("p (k h) -> p k h", k=NKT))
                nc.gpsimd.dma_start(
                    out=pt_d_sb, in_=pt_d[:, :].rearrange("p (r s) -> p r s", r=8))
            pt_d_sb = patt

            KT = big.tile([128, S], BF16, tag="KT")     # K^T [h, k]
            QT = big.tile([128, NQ], BF16, tag="QT")    # Q^T [h, q]

            # ---- PE/HAM warmup: back-to-back dummy matmuls ----
            with tc.tile_pool(name="warm", bufs=2, space="PSUM") as warm:
                for _ in range(14):
                    wp = warm.tile([128, 128], F32)
                    nc.tensor.matmul(wp, ident, ones, start=True, stop=True)

            # ---- interleaved projections + attention ----
            with (
                tc.tile_pool(name="vt_sb", bufs=1) as vt_sb_pool,
                tc.tile_pool(name="xp", bufs=16) as xpp,
                tc.tile_pool(name="x8", bufs=8) as x8p,
                tc.tile_pool(name="pp", bufs=2, space="PSUM") as pp,
                tc.tile_pool(name="sp", bufs=2, space="PSUM") as sp,
                tc.tile_pool(name="op", bufs=1, space="PSUM") as op,
                tc.tile_pool(name="dp", bufs=1, space="PSUM") as dp,
                tc.tile_pool(name="osb", bufs=2) as osb,
                tc.tile_pool(name="rp", bufs=2) as rp,
            ):
                VT = vt_sb_pool.tile([128, S], BF16, tag="VT")
                v_t = [None] * NKT
                kv_ps = {}
                q_ps = {}
                x_sb = {}   # (sc, half) -> SBUF tile [128, 4, 512] bf16
                x8_sb = {}  # sc -> SBUF tile [128, 4, 2, 512] e3m4

                def load_x(sc, dh):
                    xt = xpp.tile([128, 4, 512], BF16, tag="xt")
                    lo = sc * (DC * 512) + dh * (4 * 512)
                    nc.sync.dma_start(
                        out=xt,
                        in_=xp_d[:, lo:lo + 4 * 512].rearrange(
                            "p (c s) -> p c s", c=4))
                    x_sb[(sc, dh)] = xt

                def load_x8(sc):
                    xt = x8p.tile([128, 4, 2, 512], FP8, tag="xt8")
                    lo = sc * (DC * 512)
                    nc.sync.dma_start(
                        out=xt,
                        in_=xp8_d[:, lo:lo + DC * 512].rearrange(
                            "p (c o s) -> p c o s", c=4, o=2))
                    x8_sb[sc] = xt

                def k_project(kpsum, sc, hi_prec):
                    for dh in range(2):
                        xt = x_sb[(sc, dh)]
                        for dcl in range(4):
                            dc = dh * 4 + dcl
                            nc.tensor.matmul(
                                kpsum, w_sb["k16"][:, dc, :],
                                xt[:, dcl, :],
                                start=(dc == 0), stop=(dc == DC - 1))

                def k_evict(kpsum, sc, hi_prec):
                    nc.vector.tensor_scalar_add(
                        KT[:, sc * 512:(sc + 1) * 512], kpsum, b_sb["k"])

                def v_project(vpsum, sc):
                    # scores are fp8-sensitive but V is not: value errors
                    # enter the output linearly and average over the softmax,
                    # except for the short-context rows which only read
                    # v-tiles 0-7 -> s-chunks 0,1 project in bf16, the rest
                    # via fp8 DoubleRow
                    if sc < 2:
                        for dh in range(2):
                            xt = x_sb[(sc, dh)]
                            for dcl in range(4):
                                dc = dh * 4 + dcl
                                nc.tensor.matmul(
                                    vpsum, w_sb["v"][:, dc, :], xt[:, dcl, :],
                                    start=(dc == 0), stop=(dc == DC - 1))
                    else:
                        x8t = x8_sb[sc]
                        for dcp in range(4):
                            nc.tensor.matmul(
                                vpsum, w_sb["v8"][:, dcp, :, :],
                                x8t[:, dcp, :, :],
                                start=(dcp == 0), stop=(dcp == 3),
                                perf_mode=mybir.MatmulPerfMode.DoubleRow)

                def v_evict(vpsum, sc):
                    if sc < 2:
                        nc.vector.tensor_scalar_add(
                            VT[:, sc * 512:(sc + 1) * 512], vpsum, b_sb["v"])
                    else:
                        nc.vector.tensor_scalar(
                            VT[:, sc * 512:(sc + 1) * 512], vpsum,
                            1.0 / WSCALE, b_sb["v"],
                            mybir.AluOpType.mult, mybir.AluOpType.add)

                def project_sc(sc):
                    hi = sc < 2
                    kpsum = pp.tile([128, 512], F32, tag="pj")
                    vpsum = pp.tile([128, 512], F32, tag="pj")
                    k_project(kpsum, sc, hi)
                    v_project(vpsum, sc)
                    k_evict(kpsum, sc, hi)
                    v_evict(vpsum, sc)
                    for kt in range(4 * sc, 4 * sc + 4):
                        tpsum = pp.tile([128, 512], F32, tag="pj")
                        tview = tpsum.bitcast(BF16)[:, 0:128]
                        nc.tensor.transpose(
                            tview, VT[:, kt * 128:(kt + 1) * 128], ident)
                        vt = vtiles.tile([128, H], BF16, tag="v_t")
                        # fold the 0/1 key-padding mask into V^T rows
                        nc.vector.tensor_scalar_mul(vt, tview, vm[:, kt:kt + 1])
                        v_t[kt] = vt

                def owned_cols16(xt, dcl):
                    # bf16 moving AP over owned subtiles (positions 0, 2)
                    row = xt[:, dcl, :]
                    return bass.AP(
                        tensor=row.tensor, offset=row.offset,
                        ap=[list(row.ap[0]), [256, 2], [1, 128]])

                def q_project(qpsum, j):
                    for e in range(2):
                        for dh in range(2):
                            xt = x_sb[(2 * j + e, dh)]
                            for dcl in range(4):
                                dc = dh * 4 + dcl
                                nc.tensor.matmul(
                                    qpsum[:, e * 256:(e + 1) * 256],
                                    w_sb["q16"][:, dc, :],
                                    owned_cols16(xt, dcl),
                                    start=(dc == 0), stop=(dc == DC - 1))

                def q_evict(j):
                    nc.vector.tensor_scalar_add(
                        QT[:, j * 512:(j + 1) * 512], q_ps[j], b_sb["q"])

                def project_q(j):
                    qpsum = pp.tile([128, 512], F32, tag="pj")
                    q_ps[j] = qpsum
                    q_project(qpsum, j)
                    q_evict(j)

                pending = []

                def drain(n):
                    for _ in range(min(n, len(pending))):
                        pending.pop(0)()

                def attention(j, rate=2):
                    n_kt = 8 * j + 8
                    n_pr = n_kt // 2
                    outp = op.tile([128, 512], F32)
                    denp = dp.tile([128, 512], F32)
                    pts = [None] * n_pr

                    def acol(pr):
                        # causal dead-prefix: diagonal pair w of the chunk
                        # has its first w*128 query columns fully masked
                        # (valid for both parities)
                        return 128 * max(0, pr - 4 * j)

                    def score_exp(pr):
                        a = acol(pr)
                        spair = sp.tile([128, 2, 512], F32)
                        for o in range(2):
                            kt = 2 * pr + o
                            nc.tensor.matmul(
                                spair[:, o, a:512],
                                KT[:, kt * 128:(kt + 1) * 128],
                                QT[:, j * 512 + a:(j + 1) * 512],
                                start=True, stop=True)
                        pt = ptp.tile([128, 2, 512], BF16, tag="pt")
                        # bias -1 keeps exp within fp8e4 range (max score
                        # ~5.5 sigma); it scales num and den equally so the
                        # softmax ratio is unchanged
                        nc.scalar.activation(
                            pt[:, :, a:512], spair[:, :, a:512],
                            mybir.ActivationFunctionType.Exp, bias=neg1)
                        if 2 * pr >= 8 * j:
                            r = 2 * pr - 8 * j
                            pt2 = pt2p.tile([128, 2, 512], BF16, tag="pt2")
                            nc.vector.tensor_mul(
                                pt2[:, :, a:512], pt[:, :, a:512],
                                patt[:, r:r + 2, a:512])
                            pts[pr] = pt2
                        else:
                            pts[pr] = pt

                    def pv(pr):
                        a = acol(pr)
                        for o in range(2):
                            kt = 2 * pr + o
                            nc.tensor.matmul(
                                outp[:, a:512], v_t[kt], pts[pr][:, o, a:512],
                                start=(kt == 0), stop=(kt == n_kt - 1))
                        for o in range(2):
                            kt = 2 * pr + o
                            nc.tensor.matmul(
                                denp[:, a:512], mrep[:, kt, :],
                                pts[pr][:, o, a:512],
                                start=(kt == 0), stop=(kt == n_kt - 1))

                    # software-pipelined: PE does scores(pr+1) while the
                    # scalar engine exps scores(pr); PV lags one step.
                    # projection work for later blocks is drained into the
                    # exp-latency bubbles between steps.
                    score_exp(0)
                    for pr in range(1, n_pr):
                        score_exp(pr)
                        drain(rate)
                        pv(pr - 1)
                        drain(rate)
                    pv(n_pr - 1)

                    r_t = rp.tile([128, 512], F32, tag="r")
                    nc.vector.reciprocal(r_t, denp)
                    o_sb = osb.tile([128, 512], F32, tag="o")
                    nc.vector.tensor_mul(o_sb, outp, r_t)
                    nc.sync.dma_start(
                        out=o_d[:, j * 512:(j + 1) * 512], in_=o_sb)

                def queue_proj(sc):
                    def k_mms(sc=sc):
                        k_project(kv_ps[sc][0], sc, sc < 2)

                    def v_mms(sc=sc):
                        v_project(kv_ps[sc][1], sc)

                    def alloc(sc=sc):
                        kv_ps[sc] = (
                            pp.tile([128, 512], F32, tag="pj", name="kpsum"),
                            pp.tile([128, 512], F32, tag="pj", name="vpsum"))
                    pending.append(alloc)
                    pending.append(k_mms)
                    pending.append(v_mms)

                    def evict(sc=sc):
                        kpsum, vpsum = kv_ps[sc]
                        k_evict(kpsum, sc, sc < 2)
                        v_evict(vpsum, sc)
                    pending.append(evict)

                    def transp(kt):
                        tpsum = pp.tile([128, 512], F32, tag="pj")
                        tview = tpsum.bitcast(BF16)[:, 0:128]
                        nc.tensor.transpose(
                            tview, VT[:, kt * 128:(kt + 1) * 128], ident)
                        vt = vtiles.tile([128, H], BF16, tag="v_t")
                        nc.vector.tensor_scalar_mul(vt, tview, vm[:, kt:kt + 1])
                        v_t[kt] = vt
                    for kt in range(4 * sc, 4 * sc + 4):
                        pending.append(lambda kt=kt: transp(kt))

                def queue_q(j):
                    def alloc(j=j):
                        q_ps[j] = pp.tile(
                            [128, 512], F32, tag="pj", name="qpsum")

                    def q_mms(j=j):
                        q_project(q_ps[j], j)

                    def evict(j=j):
                        q_evict(j)
                    pending.append(alloc)
                    pending.append(q_mms)
                    pending.append(evict)

                # prefetch the whole x stream (the DMA queue runs in order)
                for sc in range(2):
                    load_x(sc, 0)
                    load_x(sc, 1)
                for sc in (2, 3):
                    load_x8(sc)
                    load_x(sc, 0)
                    load_x(sc, 1)
                load_small_consts()
                for sc in range(4, SCH):
                    load_x8(sc)
                    load_x(sc, 0)
                    load_x(sc, 1)
                project_sc(0)
                project_q(0)
                project_sc(1)
                # Q projections only need x tiles, so they go FIRST in each
                # drain batch: the QT eviction clears the DVE queue early and
                # the next attention block starts without stalling.
                queue_q(1)
                for sc in (2, 3):
                    queue_proj(sc)
                attention(0)
                drain(len(pending))
                queue_q(2)
                for sc in (4, 5):
                    queue_proj(sc)
                attention(1)
                drain(len(pending))
                queue_q(3)
                for sc in (6, 7):
                    queue_proj(sc)
                attention(2)
                drain(len(pending))
                attention(3)
    nc.compile()
    return nc


_NC_CACHE = {}


def _get_program():
    if "nc" not in _NC_CACHE:
        _NC_CACHE["nc"] = _build_program()
    return _NC_CACHE["nc"]


def _sigma(p):
    # within-chunk subtile permutation: owned subtiles at positions 0, 2
    return [0, 1, 2, 3] if p == 0 else [1, 0, 3, 2]


def _make_in_maps(x, mask, Wq, bq, Wk, bk, Wv, bv):
    x = np.asarray(x, np.float32)
    mask = np.asarray(mask)
    scale = 1.0 / np.sqrt(np.float32(H))

    def pack_w(w):
        # [H,D] -> w.T [D,H] -> partition-major [128, DC*H]
        wT = np.asarray(w, np.float32).T.reshape(DC, 128, H)
        return np.ascontiguousarray(
            wT.transpose(1, 0, 2).reshape(128, DC * H).astype(BF16_NP))

    def pack_w8(w):
        # [H,D] -> WSCALE*w.T in e3m4, laid out [128, dcp(4), o(2), H]
        wT = (np.asarray(w, np.float32).T * WSCALE).clip(-200.0, 200.0)
        wT = wT.reshape(4, 2, 128, H)           # [dcp, o, dp, h]
        return np.ascontiguousarray(
            wT.transpose(2, 0, 1, 3).reshape(128, DC * H).astype(FP8_NP))

    wv8 = pack_w8(Wv)
    wq16 = pack_w(np.asarray(Wq, np.float32) * scale)
    wk16 = pack_w(Wk)
    wv = pack_w(Wv)
    bq_c = (np.asarray(bq, np.float32) * scale).reshape(H, 1).copy()
    bk_c = np.asarray(bk, np.float32).reshape(H, 1).copy()
    bv_c = np.asarray(bv, np.float32).reshape(H, 1).copy()

    # per (batch, parity) packed x^T: [dp, sc, dc, pos, sf]
    xp_cache = {}

    def pack_x(b, p):
        if (b, p) not in xp_cache:
            xb = x[b].astype(BF16_NP)               # [s, d]
            xr = xb.reshape(SCH, 4, 128, DC, 128)   # [sc, t, sf, dc, dp]
            xr = xr[:, _sigma(p), :, :, :]          # [sc, pos, sf, dc, dp]
            xp_cache[(b, p)] = np.ascontiguousarray(
                xr.transpose(4, 0, 3, 1, 2).reshape(128, SCH * DC * 512))
        return xp_cache[(b, p)]

    xp8_cache = {}

    def pack_x8(b, p):
        # [dp, sc, dcp, o, pos, sf] in e3m4 for the DoubleRow projections
        if (b, p) not in xp8_cache:
            xb = x[b].clip(-200.0, 200.0).astype(FP8_NP)
            xr = xb.reshape(SCH, 4, 128, 4, 2, 128)  # [sc,t,sf,dcp,o,dp]
            xr = xr[:, _sigma(p), :, :, :, :]
            xp8_cache[(b, p)] = np.ascontiguousarray(
                xr.transpose(5, 0, 3, 4, 1, 2).reshape(128, SCH * DC * 512))
        return xp8_cache[(b, p)]

    patt_cache = {}

    def pack_patt(p):
        # patt[kp, r, i*128+qf]: causal 0/1 for diagonal k-tile offset r
        if p not in patt_cache:
            sig = _sigma(p)
            kp = np.arange(128)[:, None, None, None]
            r = np.arange(8)[None, :, None, None]
            i = np.arange(4)[None, None, :, None]
            qf = np.arange(128)[None, None, None, :]
            koff = 4 * (r // 4) + np.array(sig)[r % 4]
            qoff = 2 * i + p
            m = (qoff > koff) | ((qoff == koff) & (qf >= kp))
            patt_cache[p] = np.ascontiguousarray(
                m.astype(BF16_NP).reshape(128, 8 * 512))
        return patt_cache[p]

    in_maps = []
    for c in range(8):
        b, p = c // 2, c % 2
        sig = np.array(_sigma(p))
        kt = np.arange(NKT)
        g_kt = 4 * (kt // 4) + sig[kt % 4]          # global tile of k-tile kt
        key_idx = g_kt[None, :] * 128 + np.arange(128)[:, None]
        kmask = (np.asarray(mask[b])[key_idx] != 0)         # [128, NKT]
        vmask = kmask.astype(np.float32)
        mrep = np.broadcast_to(
            kmask.astype(FP8_NP)[:, :, None], (128, NKT, H))
        in_maps.append({
            "xp": pack_x(b, p), "xp8": pack_x8(b, p),
            "wv8": wv8, "wv": wv, "wq16": wq16, "wk16": wk16,
            "bq": bq_c, "bk": bk_c, "bv": bv_c,
            "vmask": np.ascontiguousarray(vmask),
            "mrep": np.ascontiguousarray(mrep.reshape(128, NKT * H)),
            "patt": pack_patt(p),
        })
    return in_maps


def _install_ntff_hook():
    # the trimmed antenv package lacks axon_hooks; recreate it and wire the
    # ctypes NTFF profiling hook from trn_agent_boot so trace=True works
    import types
    if "antenv.axon_hooks" in sys.modules:
        return
    import antenv
    mod = types.ModuleType("antenv.axon_hooks")
    _hook = [None]
    mod.set_axon_ntff_profile_hook = lambda h: _hook.__setitem__(0, h)
    mod.get_axon_ntff_profile_hook = lambda: _hook[0]
    sys.modules["antenv.axon_hooks"] = mod
    antenv.axon_hooks = mod
    from trn_agent_boot.trn_boot import _ntff_profile_via_ctypes
    mod.set_axon_ntff_profile_hook(
        _ntff_profile_via_ctypes("/opt/axon/libaxon_pjrt.so"))


def run(inputs, trace=False, tmpdir=None):
    if trace:
        try:
            _install_ntff_hook()
        except Exception as e:
            print("ntff hook install failed:", e)
    nc = _get_program()
    in_maps = _make_in_maps(**inputs)
    res = bass_utils.run_bass_kernel_spmd(
        nc, in_maps, core_ids=list(range(8)), trace=trace, tmpdir=tmpdir)
    out = np.empty((B, S, H), np.float32)
    for c in range(8):
        b, p = c // 2, c % 2
        o = res.results[c]["o"]                                # [H, NQ]
        for lt in range(16):
            g = 2 * lt + p
            out[b, g * 128:(g + 1) * 128, :] = o[:, lt * 128:(lt + 1) * 128].T
    return out, res


def kernel(**inputs) -> np.ndarray:
    out, _ = run(inputs, trace=False)
    return out
